# revision 11
# baseline (speedup 1.0000x reference)
"""LorentzConv2d Trainium2 kernel.

Full-input contract: kernel(x=[8,56,56,64], kernels=[64,64]) -> [8,56,56,64].
Data-parallel over batch: one image per NeuronCore (8 cores).

Per-core algorithm (all on a zero-padded 58x58 grid, linearized p = 58*gh+gw):
  u[p,o]   = sum_c x[p,c] * g_c * kernels[o,c]   (PE matmul; g = (+1,-1..-1))
  sx[p]    = sum_{c>=1} x[p,c]                   (extra matmul column)
  D[p,o]   = acosh(max(u, 1+eps))^2 = ln(u + sqrt(u^2-1))^2   (ACT/DVE)
  G[p,d]   = <x[p], x[p+d]>_L  for the 12 positive window offsets d (DVE/GPSIMD)
  Q[l,o]   = -box3x3(D^2)[l] + 2*sum_d boxB(d)( D * shift_d(D) * G_d )[l]
  S1[l,o]  = box3x3(sx * D)[l]
  out_o    = (S1/63) / sqrt(clip(|Q|,eps))  (o>=1);  out_0 = sqrt(1 + sum out_o^2)
All box sums are banded-Toeplitz matmuls on the PE accumulating in PSUM.
"""

import os
import numpy as np

import concourse.bass as bass
import concourse.bacc as bacc
import concourse.tile as tile
from concourse import mybir
from concourse.bass_utils import run_bass_kernel_spmd

F32 = mybir.dt.float32
AF = mybir.ActivationFunctionType
OP = mybir.AluOpType

# geometry
H = W = 56
C = 64
O = 64
GH = GW = 58              # padded grid
NG = GH * GW              # 3364
NT = 27                   # pixel tiles of 128
NP = NT * 128             # 3456 compute pixels (grid + tail)
GUARD = 128               # top guard rows in padded DRAM images
NPAD = GUARD + NP + 128   # 3712 rows in xpad/dpad
ACOSH_EPS = 1e-7
EPS = 1e-8

# the 12 positive window-pair offsets (dh, dw), linear = 58*dh+dw
DELTAS = [(0, 1), (0, 2), (1, -2), (1, -1), (1, 0), (1, 1), (1, 2),
          (2, -2), (2, -1), (2, 0), (2, 1), (2, 2)]


def _interval(d):
    return range(max(-1, -1 - d), min(1, 1 - d) + 1)


def _build_passes():
    """Each pass: (name, delta_index_or_None, coeff, box_offsets, target)."""
    box33 = [58 * a + b for a in (-1, 0, 1) for b in (-1, 0, 1)]
    passes = [("diag", None, -1.0, box33, "q")]
    for di, (dh, dw) in enumerate(DELTAS):
        box = [58 * a + b for a in _interval(dh) for b in _interval(dw)]
        passes.append((f"d{di}", di, 2.0, box, "q"))
    passes.append(("s1", None, 1.0, box33, "s"))
    return passes


def _build_bands(passes):
    """Toeplitz band matrices. For pass and side j in {-1,0,1}:
    T[i, m] = coeff if (128*j + i - m) in box else 0.
    Returns (bands [NB,128,128] f32, sides: per-pass list of (j, band_index))."""
    mats = []
    sides = []
    for (_, _, coeff, box, _) in passes:
        bs = set(box)
        plist = []
        for j in (-1, 0, 1):
            T = np.zeros((128, 128), dtype=np.float32)
            for t in bs:
                # i - m = t - 128*j ; valid i in [max(0, d), 128 + min(0, d))
                d = t - 128 * j
                if -127 <= d <= 127:
                    idx = np.arange(max(0, d), 128 + min(0, d))
                    T[idx, idx - d] = coeff
            if np.any(T):
                plist.append((j, len(mats)))
                mats.append(T)
        sides.append(plist)
    return np.stack(mats), sides


PASSES = _build_passes()
BANDS, PASS_SIDES = _build_bands(PASSES)
NB = BANDS.shape[0]


def build_nc():
    nc = bacc.Bacc(None)
    x_in = nc.declare_dram_parameter("x", [H * W, C], F32, isOutput=False)
    gk_in = nc.declare_dram_parameter("gk_ext", [C, O + 1], F32, isOutput=False)
    bands_in = nc.declare_dram_parameter("bands", [NB, 128, 128], F32, isOutput=False)
    id_in = nc.declare_dram_parameter("ident", [128, 128], F32, isOutput=False)
    out_ext = nc.declare_dram_parameter("out", [H * W, O], F32, isOutput=True)

    def tiled(dram_ap, row0, ntile=NT):
        """DRAM rows [row0, row0+128*ntile) viewed as [128, ntile, 64]."""
        return dram_ap[row0:row0 + 128 * ntile, :].rearrange(
            "(t p) c -> p t c", p=128)

    with tile.TileContext(nc) as tc:
        with (
            tc.tile_pool(name="dram", bufs=1, space="DRAM") as dpool,
            tc.tile_pool(name="singles", bufs=1) as sg,
            tc.tile_pool(name="pp", bufs=1) as pp,
        ):
            xpad = dpool.tile([NPAD, C], F32)
            dpad = dpool.tile([NPAD, O], F32)
            opad = dpool.tile([NP, O], F32)

            # ---- constants into SBUF
            gk_sb = sg.tile([C, O + 1], F32)
            nc.sync.dma_start(out=gk_sb[:], in_=gk_in[:])
            id_sb = sg.tile([128, 128], F32)
            nc.sync.dma_start(out=id_sb[:], in_=id_in[:])
            bands_sb = sg.tile([128, NB, 128], F32)
            nc.sync.dma_start(out=bands_sb[:],
                              in_=bands_in.rearrange("b p m -> p b m"))

            zsb = sg.tile([128, C], F32)
            nc.vector.memset(zsb[:], 0.0)
            cneg1 = sg.tile([128, 1], F32)
            nc.vector.memset(cneg1[:], -1.0)

            # ---- zero-fill xpad / dpad with one broadcast DMA each, then
            # overwrite the grid interior (borders stay zero).
            zbc = zsb[:].unsqueeze(1).to_broadcast([128, NPAD // 128, C])
            nc.sync.dma_start(out=tiled(xpad, 0, NPAD // 128), in_=zbc)
            nc.sync.dma_start(out=tiled(dpad, 0, NPAD // 128), in_=zbc)
            g0 = GUARD
            # interior <- input image
            nc.sync.dma_start(
                out=xpad[g0 + GW:g0 + 57 * GW, :].rearrange(
                    "(h w) c -> h w c", w=GW)[:, 1:57, :],
                in_=x_in.rearrange("(h w) c -> h w c", w=W))

            # persistent fields
            x_sb = sg.tile([128, NT, C], F32)
            nc.sync.dma_start(out=x_sb[:], in_=tiled(xpad, GUARD))
            d_sb = sg.tile([128, NT, O], F32)
            sx_sb = sg.tile([128, NT], F32)
            g_all = sg.tile([128, NT, len(DELTAS)], F32)

            # ================= phase A: u, sx, dists =================
            with (
                tc.tile_pool(name="psA", bufs=2, space="PSUM") as psA,
                tc.tile_pool(name="psT", bufs=3, space="PSUM") as psT,
                tc.tile_pool(name="sbA", bufs=3) as sbA,
            ):
                groups = [list(range(g, min(g + 4, NT))) for g in range(0, NT, 4)]
                for tiles in groups:
                    gsz = len(tiles)
                    t0 = tiles[0]
                    psu = psA.tile([128, 4, O + 1], F32)
                    for i, tl in enumerate(tiles):
                        xt_ps = psT.tile([C, 128], F32)
                        nc.tensor.transpose(xt_ps[:], x_sb[:, tl, :], id_sb[:])
                        xt_sb = sbA.tile([C, 128], F32, tag="xt")
                        nc.scalar.copy(xt_sb[:], xt_ps[:])
                        nc.tensor.matmul(psu[:, i, :], xt_sb[:], gk_sb[:],
                                         start=True, stop=True)
                    um = sbA.tile([128, 4, O], F32, tag="um")
                    nc.vector.tensor_scalar_max(um[:, :gsz, :], psu[:, :gsz, 0:O],
                                                1.0 + ACOSH_EPS)
                    nc.scalar.copy(sx_sb[:, t0:t0 + gsz], psu[:, :gsz, O])
                    sq = sbA.tile([128, 4, O], F32, tag="sq")
                    nc.scalar.activation(sq[:, :gsz, :], um[:, :gsz, :], AF.Square)
                    rt = sbA.tile([128, 4, O], F32, tag="rt")
                    nc.scalar.activation(rt[:, :gsz, :], sq[:, :gsz, :], AF.Sqrt,
                                         bias=cneg1[:])
                    vv = sbA.tile([128, 4, O], F32, tag="vv")
                    nc.vector.tensor_add(vv[:, :gsz, :], um[:, :gsz, :], rt[:, :gsz, :])
                    lnv = sbA.tile([128, 4, O], F32, tag="lnv")
                    nc.scalar.activation(lnv[:, :gsz, :], vv[:, :gsz, :], AF.Ln)
                    nc.scalar.activation(d_sb[:, t0:t0 + gsz, :], lnv[:, :gsz, :],
                                         AF.Square)
                nc.sync.dma_start(out=tiled(dpad, GUARD), in_=d_sb[:])

                # ============= phase B: Minkowski shift products G =============
                xs_pp = [pp.tile([128, NT, C], F32, tag=f"xs{i}", name=f"xs{i}") for i in range(2)]
                tg_pp = [pp.tile([128, NT, C], F32, tag=f"tg{i}", name=f"tg{i}") for i in range(2)]
                for di, (dh, dw) in enumerate(DELTAS):
                    dlin = 58 * dh + dw
                    xs = xs_pp[di % 2]
                    tg = tg_pp[di % 2]
                    nc.sync.dma_start(out=xs[:], in_=tiled(xpad, GUARD + dlin))
                    nc.gpsimd.tensor_mul(tg[:], x_sb[:], xs[:])
                    nc.vector.tensor_reduce(g_all[:, :, di], tg[:],
                                            axis=mybir.AxisListType.X, op=OP.add)
                    nc.vector.scalar_tensor_tensor(
                        out=g_all[:, :, di], in0=tg[:, :, 0], scalar=-2.0,
                        in1=g_all[:, :, di], op0=OP.mult, op1=OP.add)

            # ============= phase C: fields + banded box matmuls =============
            with (
                tc.tile_pool(name="psQ", bufs=1, space="PSUM") as psQ,
                tc.tile_pool(name="psS", bufs=1, space="PSUM") as psS,
            ):
                ps_q = psQ.tile([128, NT, O], F32)
                ps_s = psS.tile([128, NT, O], F32)

                f_pp = [pp.tile([128, NT + 2, O], F32, tag=f"f{i}", name=f"f{i}") for i in range(2)]
                ds_pp = [pp.tile([128, NT, O], F32, tag=f"ds{i}", name=f"ds{i}") for i in range(2)]
                t2_pp = [pp.tile([128, NT, O], F32, tag=f"t2{i}", name=f"t2{i}") for i in range(2)]
                for f in f_pp:
                    nc.vector.memset(f[:, 0, :], 0.0)
                    nc.vector.memset(f[:, NT + 1, :], 0.0)

                chunks = [(0, 8), (8, 8), (16, 8), (24, 3)]
                n_writes_q = sum(len(PASS_SIDES[pi]) for pi, p in enumerate(PASSES)
                                 if p[4] == "q")
                n_writes_s = sum(len(PASS_SIDES[pi]) for pi, p in enumerate(PASSES)
                                 if p[4] == "s")
                wq = [0] * len(chunks)
                ws = [0] * len(chunks)

                for pi, (name, di, _, _, tgt_kind) in enumerate(PASSES):
                    f = f_pp[pi % 2]
                    fm = f[:, 1:NT + 1, :]
                    if name == "diag":
                        nc.scalar.activation(fm, d_sb[:], AF.Square)
                    elif name == "s1":
                        nc.vector.tensor_mul(
                            fm, d_sb[:],
                            sx_sb[:].unsqueeze(2).to_broadcast([128, NT, O]))
                    else:
                        dh, dw = DELTAS[di]
                        dlin = 58 * dh + dw
                        dsh = ds_pp[pi % 2]
                        t2 = t2_pp[pi % 2]
                        nc.sync.dma_start(out=dsh[:], in_=tiled(dpad, GUARD + dlin))
                        nc.gpsimd.tensor_mul(t2[:], d_sb[:], dsh[:])
                        nc.vector.tensor_mul(
                            fm, t2[:],
                            g_all[:, :, di].unsqueeze(2).to_broadcast([128, NT, O]))
                    tgt, wcnt, wtot = ((ps_q, wq, n_writes_q) if tgt_kind == "q"
                                       else (ps_s, ws, n_writes_s))
                    for (j, bi) in PASS_SIDES[pi]:
                        for ci, (c0, cw) in enumerate(chunks):
                            nc.tensor.matmul(
                                tgt[:, c0:c0 + cw, :],
                                bands_sb[:, bi, :],
                                f[:, 1 + c0 + j:1 + c0 + j + cw, :],
                                start=(wcnt[ci] == 0),
                                stop=(wcnt[ci] == wtot - 1),
                                skip_group_check=True)
                            wcnt[ci] += 1

                # ================= phase D: normalize & emit =================
                ac = pp.tile([128, NT, O], F32)
                nc.scalar.activation(ac[:], ps_q[:], AF.Abs)
                cl = pp.tile([128, NT, O], F32)
                nc.vector.tensor_scalar_max(cl[:], ac[:], EPS)
                lnc = pp.tile([128, NT, O], F32)
                nc.scalar.activation(lnc[:], cl[:], AF.Ln)
                rr = pp.tile([128, NT, O], F32)
                nc.scalar.activation(rr[:], lnc[:], AF.Exp, scale=-0.5)
                osb = pp.tile([128, NT, O], F32)
                nc.vector.scalar_tensor_tensor(
                    out=osb[:], in0=ps_s[:], scalar=1.0 / 63.0, in1=rr[:],
                    op0=OP.mult, op1=OP.mult)
                s2 = pp.tile([128, NT, O - 1], F32)
                nc.scalar.activation(s2[:], osb[:, :, 1:O], AF.Square)
                red = pp.tile([128, NT], F32)
                nc.vector.tensor_reduce(red[:], s2[:], axis=mybir.AxisListType.X,
                                        op=OP.add)
                nc.scalar.activation(osb[:, :, 0], red[:], AF.Sqrt, bias=1.0)
                nc.sync.dma_start(out=tiled(opad, 0), in_=osb[:])

            # interior extraction (DRAM -> DRAM)
            nc.sync.dma_start(
                out=out_ext.rearrange("(h w) c -> h w c", w=W),
                in_=opad[GW:57 * GW, :].rearrange(
                    "(h w) c -> h w c", w=GW)[:, 1:57, :])
    nc.finalize()
    return nc


_NC_CACHE = None


def _get_nc():
    global _NC_CACHE
    if _NC_CACHE is None:
        _NC_CACHE = build_nc()
    return _NC_CACHE


def host_consts(kernels):
    # u = -l_inner(x,k) = x0*k0 - sum_{c>=1} x_c*k_c ; col O is sum_{c>=1} x_c
    gk_ext = np.zeros((C, O + 1), dtype=np.float32)
    gk_ext[:, :O] = kernels.astype(np.float32).T
    gk_ext[1:, :O] *= -1.0
    gk_ext[1:, O] = 1.0
    return gk_ext


def kernel(x, kernels):
    x = np.asarray(x, dtype=np.float32)
    kernels = np.asarray(kernels, dtype=np.float32)
    B = x.shape[0]
    assert x.shape == (B, H, W, C) and B == 8, x.shape
    gk_ext = np.ascontiguousarray(host_consts(kernels))
    ident = np.eye(128, dtype=np.float32)
    nc = _get_nc()
    in_maps = [{
        "x": np.ascontiguousarray(x[i].reshape(H * W, C)),
        "gk_ext": gk_ext,
        "bands": np.ascontiguousarray(BANDS),
        "ident": ident,
    } for i in range(8)]
    res = run_bass_kernel_spmd(nc, in_maps, core_ids=list(range(8)),
                               trace=bool(int(os.environ.get("KTRACE", "0"))))
    if res.exec_time_ns is not None:
        print(f"HW exec time: {res.exec_time_ns} ns")
    out = np.stack([res.results[i]["out"].reshape(H, W, O) for i in range(8)])
    return out.astype(np.float32)


# revision 13
# speedup vs baseline: 1.3033x; 1.3033x over previous
"""LorentzConv2d Trainium2 kernel (v2: bf16 box matmuls, batched field ops).

Full-input contract: kernel(x=[8,56,56,64], kernels=[64,64]) -> [8,56,56,64].
Data-parallel over batch: one image per NeuronCore (8 cores).

Per-core algorithm (all on a zero-padded 58x58 grid, linearized p = 58*gh+gw):
  u[p,o]   = sum_c x[p,c] * g_c * kernels[o,c]   (PE matmul; g = (+1,-1..-1))
  sx[p]    = sum_{c>=1} x[p,c]                   (extra matmul column)
  D[p,o]   = acosh(max(u, 1+eps))^2 = ln(u + sqrt(u^2-1))^2   (ACT/DVE)
  G[p,d]   = <x[p], x[p+d]>_L  for the 12 positive window offsets d (DVE/GPSIMD)
  Q[l,o]   = -box3x3(D^2)[l] + 2*sum_d boxB(d)( D * shift_d(D) * G_d )[l]
  S1[l,o]  = box3x3(sx * D)[l]
  out_o    = (S1/63) / sqrt(clip(|Q|,eps))  (o>=1);  out_0 = sqrt(1 + sum out_o^2)
All box sums are banded-Toeplitz matmuls on the PE accumulating in PSUM
(bf16 fields/bands, fp32 accumulation; band values 0/±1/2 are bf16-exact).
"""

import os
import numpy as np

import concourse.bass as bass
import concourse.bacc as bacc
import concourse.tile as tile
from concourse import mybir
from concourse.bass_utils import run_bass_kernel_spmd

F32 = mybir.dt.float32
BF16 = mybir.dt.bfloat16
AF = mybir.ActivationFunctionType
OP = mybir.AluOpType

# geometry
H = W = 56
C = 64
O = 64
GH = GW = 58              # padded grid
NG = GH * GW              # 3364
NT = 27                   # pixel tiles of 128
NP = NT * 128             # 3456 compute pixels (grid + tail)
GUARD = 128               # top guard rows in padded DRAM images
NPAD = GUARD + NP + 128   # 3712 rows in xpad/dpad
ACOSH_EPS = 1e-7
EPS = 1e-8

# the 12 positive window-pair offsets (dh, dw), linear = 58*dh+dw
DELTAS = [(0, 1), (0, 2), (1, -2), (1, -1), (1, 0), (1, 1), (1, 2),
          (2, -2), (2, -1), (2, 0), (2, 1), (2, 2)]
ND = len(DELTAS)


def _interval(d):
    return range(max(-1, -1 - d), min(1, 1 - d) + 1)


def _build_passes():
    """Each pass: (name, delta_index_or_None, coeff, box_offsets, target)."""
    box33 = [58 * a + b for a in (-1, 0, 1) for b in (-1, 0, 1)]
    passes = [("diag", None, -1.0, box33, "q")]
    for di, (dh, dw) in enumerate(DELTAS):
        box = [58 * a + b for a in _interval(dh) for b in _interval(dw)]
        passes.append((f"d{di}", di, 2.0, box, "q"))
    passes.append(("s1", None, 1.0, box33, "s"))
    return passes


def _build_bands(passes):
    """Toeplitz band matrices. For pass and side j in {-1,0,1}:
    T[i, m] = coeff if (128*j + i - m) in box else 0.
    Returns (bands [NB,128,128], sides: per-pass list of (j, band_index))."""
    mats = []
    sides = []
    for (_, _, coeff, box, _) in passes:
        bs = set(box)
        plist = []
        for j in (-1, 0, 1):
            T = np.zeros((128, 128), dtype=np.float32)
            for t in bs:
                d = t - 128 * j
                if -127 <= d <= 127:
                    idx = np.arange(max(0, d), 128 + min(0, d))
                    T[idx, idx - d] = coeff
            if np.any(T):
                plist.append((j, len(mats)))
                mats.append(T)
        sides.append(plist)
    return np.stack(mats), sides


PASSES = _build_passes()
BANDS, PASS_SIDES = _build_bands(PASSES)
NB = BANDS.shape[0]


def build_nc():
    nc = bacc.Bacc(None)
    x_in = nc.declare_dram_parameter("x", [H * W, C], F32, isOutput=False)
    gk_in = nc.declare_dram_parameter("gk_ext", [C, O + 1], F32, isOutput=False)
    bands_in = nc.declare_dram_parameter("bands", [NB, 128, 128], BF16,
                                         isOutput=False)
    id_in = nc.declare_dram_parameter("ident", [128, 128], F32, isOutput=False)
    out_ext = nc.declare_dram_parameter("out", [H * W, O], F32, isOutput=True)

    def tiled(dram_ap, row0, ntile=NT):
        """DRAM rows [row0, row0+128*ntile) viewed as [128, ntile, 64]."""
        return dram_ap[row0:row0 + 128 * ntile, :].rearrange(
            "(t p) c -> p t c", p=128)

    with tile.TileContext(nc) as tc:
        with (
            tc.tile_pool(name="dram", bufs=1, space="DRAM") as dpool,
            tc.tile_pool(name="singles", bufs=1) as sg,
            tc.tile_pool(name="pp", bufs=1) as pp,
        ):
            xpad = dpool.tile([NPAD, C], F32)
            dpad16 = dpool.tile([NPAD, O], BF16)
            opad = dpool.tile([NP, O], F32)

            # ---- constants into SBUF
            gk_sb = sg.tile([C, O + 1], F32)
            nc.sync.dma_start(out=gk_sb[:], in_=gk_in[:])
            id_sb = sg.tile([128, 128], F32)
            nc.sync.dma_start(out=id_sb[:], in_=id_in[:])
            bands_sb = sg.tile([128, NB, 128], BF16)
            nc.sync.dma_start(out=bands_sb[:],
                              in_=bands_in.rearrange("b p m -> p b m"))

            zsb = sg.tile([128, C], F32)
            nc.vector.memset(zsb[:], 0.0)
            zsb16 = sg.tile([128, C], BF16)
            nc.vector.memset(zsb16[:], 0.0)
            cneg1 = sg.tile([128, 1], F32)
            nc.vector.memset(cneg1[:], -1.0)

            # ---- zero-fill pads (one broadcast DMA each), interior overwrite
            nc.sync.dma_start(
                out=tiled(xpad, 0, NPAD // 128),
                in_=zsb[:].unsqueeze(1).to_broadcast([128, NPAD // 128, C]))
            nc.sync.dma_start(
                out=tiled(dpad16, 0, NPAD // 128),
                in_=zsb16[:].unsqueeze(1).to_broadcast([128, NPAD // 128, O]))
            g0 = GUARD
            nc.sync.dma_start(
                out=xpad[g0 + GW:g0 + 57 * GW, :].rearrange(
                    "(h w) c -> h w c", w=GW)[:, 1:57, :],
                in_=x_in.rearrange("(h w) c -> h w c", w=W))

            # persistent fields
            x_sb = sg.tile([128, NT, C], F32)
            nc.sync.dma_start(out=x_sb[:], in_=tiled(xpad, GUARD))
            gx_sb = sg.tile([128, NT, C], F32)   # x with channel 0 negated
            d_sb = sg.tile([128, NT, O], F32)
            d16 = sg.tile([128, NT, O], BF16)
            sx_sb = sg.tile([128, NT], F32)
            sx16 = sg.tile([128, NT], BF16)
            g_d = [sg.tile([128, NT], F32, tag=f"g_{i}", name=f"g_{i}")
                   for i in range(ND)]
            g16_d = [sg.tile([128, NT], BF16, tag=f"g16_{i}", name=f"g16_{i}")
                     for i in range(ND)]
            xT = sg.tile([64, NT, 128], F32)

            # gx = x with channel 0 negated (for Minkowski products)
            nc.vector.tensor_copy(gx_sb[:], x_sb[:])
            nc.vector.tensor_scalar_mul(gx_sb[:, :, 0], gx_sb[:, :, 0], -1.0)

            # ================= phase A: u, sx, dists =================
            with (
                tc.tile_pool(name="psA", bufs=1, space="PSUM") as psA,
                tc.tile_pool(name="psT", bufs=3, space="PSUM") as psT,
            ):
                # 4 PSUM pieces of <=7 tiles each so every matmul output stays
                # inside one 2KB PSUM bank (7*65*4B = 1820B)
                ugroups = [(0, 7), (7, 7), (14, 7), (21, 6)]
                psu_g = [psA.tile([128, 7, O + 1], F32, tag=f"psu{i}",
                                  name=f"psu{i}") for i in range(4)]
                for gi, (t0, tn) in enumerate(ugroups):
                    for i in range(tn):
                        tl = t0 + i
                        xt_ps = psT.tile([C, 128], F32)
                        nc.tensor.transpose(xt_ps[:], x_sb[:, tl, :], id_sb[:])
                        nc.scalar.copy(xT[:, tl, :], xt_ps[:])
                        nc.tensor.matmul(psu_g[gi][:, i, :], xT[:, tl, :],
                                         gk_sb[:], start=True, stop=True)
                # batched dists pipeline over all tiles
                um = pp.tile([128, NT, O], F32)
                for gi, (t0, tn) in enumerate(ugroups):
                    nc.vector.tensor_scalar_max(um[:, t0:t0 + tn, :],
                                                psu_g[gi][:, :tn, 0:O],
                                                1.0 + ACOSH_EPS)
                    nc.scalar.copy(sx_sb[:, t0:t0 + tn], psu_g[gi][:, :tn, O])
                nc.vector.tensor_copy(sx16[:], sx_sb[:])
                sq = pp.tile([128, NT, O], F32)
                nc.scalar.activation(sq[:], um[:], AF.Square)
                rt = pp.tile([128, NT, O], F32)
                nc.scalar.activation(rt[:], sq[:], AF.Sqrt, bias=cneg1[:])
                vv = pp.tile([128, NT, O], F32)
                nc.vector.tensor_add(vv[:], um[:], rt[:])
                lnv = pp.tile([128, NT, O], F32)
                nc.scalar.activation(lnv[:], vv[:], AF.Ln)
                nc.scalar.activation(d_sb[:], lnv[:], AF.Square)
                nc.scalar.copy(d16[:], d_sb[:])
                nc.sync.dma_start(out=tiled(dpad16, GUARD), in_=d16[:])

                # ============= phase B: Minkowski shift products G =============
                # G_d[p] = sum_c gx[p,c] * x[p+d,c]; mult split DVE/GPSIMD,
                # reduce on DVE.
                SPL = 13
                xs_pp = [pp.tile([128, NT, C], F32, tag=f"xs{i}", name=f"xs{i}")
                         for i in range(2)]
                tg_pp = [pp.tile([128, NT, C], F32, tag=f"tg{i}", name=f"tg{i}")
                         for i in range(2)]
                for di, (dh, dw) in enumerate(DELTAS):
                    dlin = 58 * dh + dw
                    xs = xs_pp[di % 2]
                    tg = tg_pp[di % 2]
                    nc.sync.dma_start(out=xs[:], in_=tiled(xpad, GUARD + dlin))
                    nc.gpsimd.tensor_mul(tg[:, :SPL, :], gx_sb[:, :SPL, :],
                                         xs[:, :SPL, :])
                    nc.vector.tensor_mul(tg[:, SPL:, :], gx_sb[:, SPL:, :],
                                         xs[:, SPL:, :])
                    nc.vector.tensor_reduce(g_d[di][:], tg[:],
                                            axis=mybir.AxisListType.X, op=OP.add)
                    nc.vector.tensor_copy(g16_d[di][:], g_d[di][:])

            # ============= phase C: fields + banded box matmuls =============
            with (
                tc.tile_pool(name="psQ", bufs=1, space="PSUM") as psQ,
                tc.tile_pool(name="psS", bufs=1, space="PSUM") as psS,
            ):
                ps_q = psQ.tile([128, NT, O], F32)
                ps_s = psS.tile([128, NT, O], F32)

                f_pp = [pp.tile([128, NT + 2, O], BF16, tag=f"f{i}", name=f"f{i}")
                        for i in range(2)]
                ds_pp = [pp.tile([128, NT, O], BF16, tag=f"ds{i}", name=f"ds{i}")
                         for i in range(2)]
                t2_pp = [pp.tile([128, NT, O], BF16, tag=f"t2{i}", name=f"t2{i}")
                         for i in range(2)]
                for f in f_pp:
                    nc.vector.memset(f[:, 0, :], 0.0)
                    nc.vector.memset(f[:, NT + 1, :], 0.0)

                chunks = [(0, 8), (8, 8), (16, 8), (24, 3)]
                n_writes_q = sum(len(PASS_SIDES[pi]) for pi, p in enumerate(PASSES)
                                 if p[4] == "q")
                n_writes_s = sum(len(PASS_SIDES[pi]) for pi, p in enumerate(PASSES)
                                 if p[4] == "s")
                wq = [0] * len(chunks)
                ws = [0] * len(chunks)

                for pi, (name, di, _, _, tgt_kind) in enumerate(PASSES):
                    f = f_pp[pi % 2]
                    fm = f[:, 1:NT + 1, :]
                    if name == "diag":
                        nc.scalar.activation(fm, d_sb[:], AF.Square)
                    elif name == "s1":
                        nc.vector.tensor_mul(
                            fm, d16[:],
                            sx16[:].unsqueeze(2).to_broadcast([128, NT, O]))
                    else:
                        dh, dw = DELTAS[di]
                        dlin = 58 * dh + dw
                        dsh = ds_pp[pi % 2]
                        t2 = t2_pp[pi % 2]
                        nc.sync.dma_start(out=dsh[:],
                                          in_=tiled(dpad16, GUARD + dlin))
                        nc.vector.tensor_mul(t2[:], d16[:], dsh[:])
                        nc.vector.tensor_mul(
                            fm, t2[:],
                            g16_d[di][:].unsqueeze(2).to_broadcast([128, NT, O]))
                    tgt, wcnt, wtot = ((ps_q, wq, n_writes_q) if tgt_kind == "q"
                                       else (ps_s, ws, n_writes_s))
                    for (j, bi) in PASS_SIDES[pi]:
                        for ci, (c0, cw) in enumerate(chunks):
                            nc.tensor.matmul(
                                tgt[:, c0:c0 + cw, :],
                                bands_sb[:, bi, :],
                                f[:, 1 + c0 + j:1 + c0 + j + cw, :],
                                start=(wcnt[ci] == 0),
                                stop=(wcnt[ci] == wtot - 1),
                                skip_group_check=True)
                            wcnt[ci] += 1

                # ================= phase D: normalize & emit =================
                ac = pp.tile([128, NT, O], F32)
                nc.scalar.activation(ac[:], ps_q[:], AF.Abs)
                cl = pp.tile([128, NT, O], F32)
                nc.vector.tensor_scalar_max(cl[:], ac[:], EPS)
                lnc = pp.tile([128, NT, O], F32)
                nc.scalar.activation(lnc[:], cl[:], AF.Ln)
                rr = pp.tile([128, NT, O], F32)
                nc.scalar.activation(rr[:], lnc[:], AF.Exp, scale=-0.5)
                osb = pp.tile([128, NT, O], F32)
                nc.vector.scalar_tensor_tensor(
                    out=osb[:], in0=ps_s[:], scalar=1.0 / 63.0, in1=rr[:],
                    op0=OP.mult, op1=OP.mult)
                s2 = pp.tile([128, NT, O - 1], F32)
                nc.scalar.activation(s2[:], osb[:, :, 1:O], AF.Square)
                red = pp.tile([128, NT], F32)
                nc.vector.tensor_reduce(red[:], s2[:], axis=mybir.AxisListType.X,
                                        op=OP.add)
                nc.scalar.activation(osb[:, :, 0], red[:], AF.Sqrt, bias=1.0)
                nc.sync.dma_start(out=tiled(opad, 0), in_=osb[:])

            # interior extraction (DRAM -> DRAM)
            nc.sync.dma_start(
                out=out_ext.rearrange("(h w) c -> h w c", w=W),
                in_=opad[GW:57 * GW, :].rearrange(
                    "(h w) c -> h w c", w=GW)[:, 1:57, :])
    nc.finalize()
    return nc


_NC_CACHE = None


def _get_nc():
    global _NC_CACHE
    if _NC_CACHE is None:
        _NC_CACHE = build_nc()
    return _NC_CACHE


def host_consts(kernels):
    # u = -l_inner(x,k) = x0*k0 - sum_{c>=1} x_c*k_c ; col O is sum_{c>=1} x_c
    gk_ext = np.zeros((C, O + 1), dtype=np.float32)
    gk_ext[:, :O] = kernels.astype(np.float32).T
    gk_ext[1:, :O] *= -1.0
    gk_ext[1:, O] = 1.0
    return gk_ext


def kernel(x, kernels):
    import ml_dtypes
    x = np.asarray(x, dtype=np.float32)
    kernels = np.asarray(kernels, dtype=np.float32)
    B = x.shape[0]
    assert x.shape == (B, H, W, C) and B == 8, x.shape
    gk_ext = np.ascontiguousarray(host_consts(kernels))
    ident = np.eye(128, dtype=np.float32)
    bands16 = np.ascontiguousarray(BANDS.astype(ml_dtypes.bfloat16))
    nc = _get_nc()
    in_maps = [{
        "x": np.ascontiguousarray(x[i].reshape(H * W, C)),
        "gk_ext": gk_ext,
        "bands": bands16,
        "ident": ident,
    } for i in range(8)]
    res = run_bass_kernel_spmd(nc, in_maps, core_ids=list(range(8)),
                               trace=bool(int(os.environ.get("KTRACE", "0"))))
    if res.exec_time_ns is not None:
        print(f"HW exec time: {res.exec_time_ns} ns")
    out = np.stack([res.results[i]["out"].reshape(H, W, O) for i in range(8)])
    return out.astype(np.float32)
